# revision 1
# baseline (speedup 1.0000x reference)
"""Trainium2 Bass kernel for DiscreteMMSE sequential posterior prediction.

Math (per batch b, point n, task t):
    diff[n,t] = data[n,:] @ W[:,t] - targets[n]          (one K=65 matmul)
    lp[n,t]   = diff[n,t]**2                              (ACT Square)
    beta[n,t] = 0.5 * sum_{j<n} lp[j,t]                   (= -alpha)
    w[n,:]    = softmax(-beta[n,:]) over t
    preds[n]  = sum_t w[n,t]*diff[n,t] + targets[n]

Layout: points on PSUM partitions (tiles of 127 points + carry row 127),
tasks on the free axis (4 chunks of 512).  The exclusive cumsum over
points is one 128x128 triangular matmul per tile; the cross-tile carry
is beta row 127, copied PSUM->SBUF into rows 96..127 of the NEXT tile's
lp tile (rows 96..126 are overwritten by that tile's square - only row
127 survives as the carry).  Per-chunk softmax stabilization by the
row-min of beta (tensor_reduce), chunks merged with exp-weights.

Sharding: data-parallel over batch, 4 batches per core on 8 cores.
Host pre-transposes data and augments W with a -1 row so one matmul
produces diff directly.
"""

import os
import sys

import numpy as np

try:
    import concourse.bass as bass  # noqa: F401
except ImportError:
    for _p in ("/opt/trn_rl_repo", "/root/.axon_site/_ro/trn_rl_repo"):
        if os.path.isdir(_p) and _p not in sys.path:
            sys.path.insert(0, _p)
    import concourse.bass as bass  # noqa: F401

from contextlib import ExitStack

import concourse.tile as tile
from concourse import bacc, mybir
from concourse.bass_utils import run_bass_kernel_spmd

B, N, D, T = 32, 1024, 64, 2048
NCORES = 8
BL = B // NCORES           # batches per core
PTS = 127                  # points per tile (row 127 = carry)
NT = (N + PTS - 1) // PTS  # 9 point-tiles per batch
CH = 512                   # task-chunk width (1 PSUM bank)
NCH = T // CH              # 4 chunks
F32 = mybir.dt.float32
ALU = mybir.AluOpType
AF = mybir.ActivationFunctionType


def build_kernel_program():
    nc = bacc.Bacc(
        "TRN2", target_bir_lowering=False, debug=False, num_devices=NCORES
    )
    dta = nc.dram_tensor("dta", [BL, 2 * D, N], F32, kind="ExternalInput").ap()
    wa = nc.dram_tensor("wa", [2 * D, T], F32, kind="ExternalInput").ap()
    lmat = nc.dram_tensor("lmat", [128, 128], F32, kind="ExternalInput").ap()
    tcol = nc.dram_tensor("tcol", [BL, N], F32, kind="ExternalInput").ap()
    out = nc.dram_tensor("out", [BL, N], F32, kind="ExternalOutput").ap()

    with tile.TileContext(nc) as tc, ExitStack() as ctx:
        _emit(ctx, tc, out, dta, wa, lmat, tcol)
    nc.compile()
    return nc


def _emit(ctx, tc, out, dta, wa, lmat, tcol):
    nc = tc.nc
    consts = ctx.enter_context(tc.tile_pool(name="consts", bufs=1))

    wa_sb = consts.tile([2 * D, T], F32, tag="wa")
    nc.sync.dma_start(wa_sb[:], wa[:, :])
    l_sb = consts.tile([128, 128], F32, tag="lmat")
    nc.sync.dma_start(l_sb[:], lmat[:, :])
    dta_sb = []
    for b in range(BL):
        t_ = consts.tile([2 * D, N], F32, tag=f"dta{b}")
        nc.sync.dma_start(t_[:], dta[b])
        dta_sb.append(t_)

    lp_pool = ctx.enter_context(tc.tile_pool(name="lp", bufs=3 * NCH))
    e_pool = ctx.enter_context(tc.tile_pool(name="e", bufs=NCH + 4))
    scr_pool = ctx.enter_context(tc.tile_pool(name="scr", bufs=NCH + 4))
    sm_pool = ctx.enter_context(tc.tile_pool(name="small", bufs=6))
    tcp_pool = ctx.enter_context(tc.tile_pool(name="tcp", bufs=6))
    pd_pool = ctx.enter_context(tc.tile_pool(name="pdiff", bufs=5, space="PSUM"))
    pb_pool = ctx.enter_context(tc.tile_pool(name="pbeta", bufs=3, space="PSUM"))

    for b in range(BL):
        lp_cur = None
        for k in range(NT):
            off = k * PTS
            m = min(PTS, N - off)


            if k == 0:
                lp_cur = [lp_pool.tile([128, CH], F32, tag="lp", name=f"lp0_{b}_{c}") for c in range(NCH)]
                for c in range(NCH):
                    # zero carry row; square overwrites rows 96..126
                    nc.vector.memset(lp_cur[c][96:128, :], 0.0)
            lp_next = (
                [
                    lp_pool.tile(
                        [128, CH], F32, tag="lp", name=f"lp_{b}_{k + 1}_{c}"
                    )
                    for c in range(NCH)
                ]
                if k + 1 < NT
                else None
            )

            ntct = tcp_pool.tile([128, 1], F32, tag="ntct")
            nc.sync.dma_start(
                ntct[0:m, :], tcol[b : b + 1, off : off + m].rearrange("a b -> b a")
            )

            mins = sm_pool.tile([128, NCH], F32, tag="mins")
            dens = sm_pool.tile([128, NCH], F32, tag="dens")
            nums = sm_pool.tile([128, NCH], F32, tag="nums")

            if m < PTS:
                # partial tile: rows m..126 of lp feed the L matmul too
                for c in range(NCH):
                    nc.vector.memset(lp_cur[c][0:96, :], 0.0)

            pds = []
            for c in range(NCH):
                # XW via K=64 matmul; chunk parity picks PE row-group 0/64
                # so consecutive chunks' matmuls run concurrently
                g = (c % 2) * D
                pd = pd_pool.tile([128, CH], F32, tag="pd")
                nc.tensor.matmul(
                    pd[0:m, :],
                    lhsT=dta_sb[b][g : g + D, off : off + m],
                    rhs=wa_sb[g : g + D, c * CH : (c + 1) * CH],
                    start=True,
                    stop=True,
                )
                pds.append(pd)
            for c in range(NCH):
                lp = lp_cur[c]
                pd = pds[c]
                # lp = (XW - targets)^2 via the Square pre-bias
                nc.scalar.activation(
                    lp[0:m, :], pd[0:m, :], AF.Square, bias=ntct[0:m, :], scale=1.0
                )

                # beta = L^T @ lp : exclusive 0.5*cumsum + carry
                pb = pb_pool.tile([128, CH], F32, tag="pb")
                nc.tensor.matmul(
                    pb[:, :], lhsT=l_sb[:, :], rhs=lp[:, :], start=True, stop=True
                )

                # per-row chunk min for softmax stabilization
                nc.vector.tensor_reduce(
                    mins[:, c : c + 1], pb[:], axis=mybir.AxisListType.X, op=ALU.min
                )

                # carry: beta rows 96..127 -> next lp rows 96..127 (only row
                # 127 survives; alternate engines to balance load)
                if lp_next is not None:
                    if c % 2 == 0:
                        nc.scalar.copy(lp_next[c][96:128, :], pb[96:128, :])
                    else:
                        nc.vector.tensor_copy(lp_next[c][96:128, :], pb[96:128, :])

                # e = exp(min_c - beta); den_c = sum_t e (ACT accumulate)
                e = e_pool.tile([128, CH], F32, tag="e")
                nc.scalar.activation(
                    e[:],
                    pb[:],
                    AF.Exp,
                    bias=mins[:, c : c + 1],
                    scale=-1.0,
                    accum_out=dens[:, c : c + 1],
                )

                # num_c = sum_t e * XW (STT accumulate)
                scr = scr_pool.tile([128, CH], F32, tag="scr")
                nc.vector.scalar_tensor_tensor(
                    out=scr[0:m, :],
                    in0=e[0:m, :],
                    scalar=1.0,
                    in1=pds[c][0:m, :],
                    op0=ALU.mult,
                    op1=ALU.mult,
                    accum_out=nums[0:m, c : c + 1],
                )

            lp_cur = lp_next

            # merge chunks: M* = min_c mins ; s_c = exp(M* - mins_c)
            mstar = sm_pool.tile([128, 1], F32, tag="mstar")
            nc.vector.tensor_reduce(
                mstar[0:m], mins[0:m], axis=mybir.AxisListType.X, op=ALU.min
            )
            s = sm_pool.tile([128, NCH], F32, tag="s")
            nc.scalar.activation(
                s[0:m], mins[0:m], AF.Exp, bias=mstar[0:m], scale=-1.0
            )

            mscr = sm_pool.tile([128, NCH], F32, tag="mscr")
            dent = sm_pool.tile([128, 1], F32, tag="dent")
            nc.vector.scalar_tensor_tensor(
                out=mscr[0:m], in0=dens[0:m], scalar=1.0, in1=s[0:m],
                op0=ALU.mult, op1=ALU.mult, accum_out=dent[0:m],
            )
            mscr2 = sm_pool.tile([128, NCH], F32, tag="mscr2")
            numt = sm_pool.tile([128, 1], F32, tag="numt")
            nc.vector.scalar_tensor_tensor(
                out=mscr2[0:m], in0=nums[0:m], scalar=1.0, in1=s[0:m],
                op0=ALU.mult, op1=ALU.mult, accum_out=numt[0:m],
            )

            rec = sm_pool.tile([128, 1], F32, tag="rec")
            nc.vector.reciprocal(rec[0:m], dent[0:m])
            pcol = sm_pool.tile([128, 1], F32, tag="pcol")
            nc.vector.scalar_tensor_tensor(
                out=pcol[0:m, :],
                in0=numt[0:m, :],
                scalar=rec[0:m, :],
                in1=numt[0:m, :],
                op0=ALU.mult,
                op1=ALU.bypass,
            )
            nc.sync.dma_start(
                out[b : b + 1, off : off + m].rearrange("a b -> b a"), pcol[0:m, :]
            )


_NC = None


def _get_nc():
    global _NC
    if _NC is None:
        _NC = build_kernel_program()
    return _NC


def make_lmat():
    j = np.arange(128)[:, None]
    n = np.arange(128)[None, :]
    L = np.where(j < n, 0.5, 0.0).astype(np.float32)
    L[127, :] = 1.0
    return L


def make_in_maps(data, targets, W):
    data = np.ascontiguousarray(data, np.float32)
    targets = np.ascontiguousarray(targets, np.float32)
    W = np.ascontiguousarray(W, np.float32)
    wa = np.concatenate([W, W], axis=0)  # rows 64..127 duplicate W
    L = make_lmat()
    in_maps = []
    for c in range(NCORES):
        db = data[c * BL : (c + 1) * BL]       # (BL, N, D)
        tb = targets[c * BL : (c + 1) * BL]    # (BL, N)
        dt_ = db.transpose(0, 2, 1)
        dta = np.concatenate([dt_, dt_], axis=1)  # (BL, 2D, N) duplicated
        in_maps.append(
            {
                "dta": np.ascontiguousarray(dta, np.float32),
                "wa": wa,
                "lmat": L,
                "tcol": np.ascontiguousarray(-tb, np.float32),
            }
        )
    return in_maps


def kernel(data, targets, W):
    nc = _get_nc()
    in_maps = make_in_maps(data, targets, W)
    res = run_bass_kernel_spmd(nc, in_maps, list(range(NCORES)))
    outs = [res.results[c]["out"] for c in range(NCORES)]
    return np.concatenate(outs, axis=0).astype(np.float32)



# revision 6
# speedup vs baseline: 1.0281x; 1.0281x over previous
"""Trainium2 Bass kernel for DiscreteMMSE sequential posterior prediction.

Math (per batch b, point n, task t):
    diff[n,t] = data[n,:] @ W[:,t] - targets[n]          (one K=65 matmul)
    lp[n,t]   = diff[n,t]**2                              (ACT Square)
    beta[n,t] = 0.5 * sum_{j<n} lp[j,t]                   (= -alpha)
    w[n,:]    = softmax(-beta[n,:]) over t
    preds[n]  = sum_t w[n,t]*diff[n,t] + targets[n]

Layout: points on PSUM partitions (tiles of 127 points + carry row 127),
tasks on the free axis (4 chunks of 512).  The exclusive cumsum over
points is one 128x128 triangular matmul per tile; the cross-tile carry
is beta row 127, copied PSUM->SBUF into rows 96..127 of the NEXT tile's
lp tile (rows 96..126 are overwritten by that tile's square - only row
127 survives as the carry).  Per-chunk softmax stabilization by the
row-min of beta (tensor_reduce), chunks merged with exp-weights.

Sharding: data-parallel over batch, 4 batches per core on 8 cores.
Host pre-transposes data and augments W with a -1 row so one matmul
produces diff directly.
"""

import os
import sys

import numpy as np

try:
    import concourse.bass as bass  # noqa: F401
except ImportError:
    for _p in ("/opt/trn_rl_repo", "/root/.axon_site/_ro/trn_rl_repo"):
        if os.path.isdir(_p) and _p not in sys.path:
            sys.path.insert(0, _p)
    import concourse.bass as bass  # noqa: F401

from contextlib import ExitStack

import concourse.tile as tile
from concourse import bacc, mybir
from concourse.bass_utils import run_bass_kernel_spmd

B, N, D, T = 32, 1024, 64, 2048
NCORES = 8
BL = B // NCORES           # batches per core
PTS = 127                  # points per tile (row 127 = carry)
NT = (N + PTS - 1) // PTS  # 9 point-tiles per batch
CH = 512                   # task-chunk width (1 PSUM bank)
NCH = T // CH              # 4 chunks
F32 = mybir.dt.float32
F32R = mybir.dt.float32r
ALU = mybir.AluOpType
AF = mybir.ActivationFunctionType


def build_kernel_program():
    nc = bacc.Bacc(
        "TRN2", target_bir_lowering=False, debug=False, num_devices=NCORES
    )
    dta = nc.dram_tensor("dta", [BL, 2 * D, N], F32R, kind="ExternalInput").ap()
    wa = nc.dram_tensor("wa", [2 * D, T], F32R, kind="ExternalInput").ap()
    lmat = nc.dram_tensor("lmat", [128, 128], F32R, kind="ExternalInput").ap()
    tcol = nc.dram_tensor("tcol", [BL, N], F32, kind="ExternalInput").ap()
    out = nc.dram_tensor("out", [BL, N], F32, kind="ExternalOutput").ap()

    with tile.TileContext(nc) as tc, ExitStack() as ctx:
        _emit(ctx, tc, out, dta, wa, lmat, tcol)
    nc.compile()
    return nc


def _emit(ctx, tc, out, dta, wa, lmat, tcol):
    nc = tc.nc
    consts = ctx.enter_context(tc.tile_pool(name="consts", bufs=1))

    wa_sb = consts.tile([2 * D, T], F32R, tag="wa")
    nc.sync.dma_start(wa_sb[:], wa[:, :])
    l_sb = consts.tile([128, 128], F32R, tag="lmat")
    nc.sync.dma_start(l_sb[:], lmat[:, :])
    dta_sb = []
    for b in range(BL):
        t_ = consts.tile([2 * D, N], F32R, tag=f"dta{b}")
        nc.sync.dma_start(t_[:], dta[b])
        dta_sb.append(t_)

    lp_pool = ctx.enter_context(tc.tile_pool(name="lp", bufs=3 * NCH))
    e_pool = ctx.enter_context(tc.tile_pool(name="e", bufs=NCH + 4))
    scr_pool = ctx.enter_context(tc.tile_pool(name="scr", bufs=NCH + 4))
    sm_pool = ctx.enter_context(tc.tile_pool(name="small", bufs=6))
    tcp_pool = ctx.enter_context(tc.tile_pool(name="tcp", bufs=6))
    pd_pool = ctx.enter_context(tc.tile_pool(name="pdiff", bufs=5, space="PSUM"))
    pb_pool = ctx.enter_context(tc.tile_pool(name="pbeta", bufs=3, space="PSUM"))

    for b in range(BL):
        lp_cur = None
        for k in range(NT):
            off = k * PTS
            m = min(PTS, N - off)


            if k == 0:
                lp_cur = [lp_pool.tile([128, CH], F32R, tag="lp", name=f"lp0_{b}_{c}") for c in range(NCH)]
                for c in range(NCH):
                    # zero carry row; square overwrites rows 96..126
                    nc.vector.memset(lp_cur[c][96:128, :].bitcast(F32), 0.0)
            lp_next = (
                [
                    lp_pool.tile(
                        [128, CH], F32R, tag="lp", name=f"lp_{b}_{k + 1}_{c}"
                    )
                    for c in range(NCH)
                ]
                if k + 1 < NT
                else None
            )

            ntct = tcp_pool.tile([128, 1], F32, tag="ntct")
            nc.sync.dma_start(
                ntct[0:m, :], tcol[b : b + 1, off : off + m].rearrange("a b -> b a")
            )

            mins = sm_pool.tile([128, NCH], F32, tag="mins")
            dens = sm_pool.tile([128, NCH], F32, tag="dens")
            nums = sm_pool.tile([128, NCH], F32, tag="nums")

            if m < PTS:
                # partial tile: rows m..126 of lp feed the L matmul too
                for c in range(NCH):
                    nc.vector.memset(lp_cur[c][0:96, :].bitcast(F32), 0.0)

            pds = []
            for c in range(NCH):
                # XW via K=64 matmul; chunk parity picks PE row-group 0/64
                # so consecutive chunks' matmuls run concurrently
                g = (c % 2) * D
                pd = pd_pool.tile([128, CH], F32, tag="pd")
                nc.tensor.matmul(
                    pd[0:m, :],
                    lhsT=dta_sb[b][g : g + D, off : off + m],
                    rhs=wa_sb[g : g + D, c * CH : (c + 1) * CH],
                    start=True,
                    stop=True,
                )
                pds.append(pd)
            for c in range(NCH):
                lp = lp_cur[c]
                pd = pds[c]
                # lp = (XW - targets)^2 via the Square pre-bias
                nc.scalar.activation(
                    lp[0:m, :], pd[0:m, :], AF.Square, bias=ntct[0:m, :], scale=1.0
                )

                # beta = L^T @ lp : exclusive 0.5*cumsum + carry
                pb = pb_pool.tile([128, CH], F32, tag="pb")
                nc.tensor.matmul(
                    pb[:, :],
                    lhsT=l_sb[:, :],
                    rhs=lp[:, :],
                    start=True,
                    stop=True,
                )

                # per-row chunk min for softmax stabilization
                nc.vector.tensor_reduce(
                    mins[:, c : c + 1], pb[:], axis=mybir.AxisListType.X, op=ALU.min
                )

                # carry: beta rows 96..127 -> next lp rows 96..127 (only row
                # 127 survives; alternate engines to balance load)
                if lp_next is not None:
                    if c % 2 == 0:
                        nc.scalar.copy(lp_next[c][96:128, :], pb[96:128, :])
                    else:
                        nc.vector.tensor_copy(lp_next[c][96:128, :], pb[96:128, :])

                # e = exp(min_c - beta); den_c = sum_t e (ACT accumulate)
                e = e_pool.tile([128, CH], F32, tag="e")
                nc.scalar.activation(
                    e[:],
                    pb[:],
                    AF.Exp,
                    bias=mins[:, c : c + 1],
                    scale=-1.0,
                    accum_out=dens[:, c : c + 1],
                )

                # num_c = sum_t e * XW (STT accumulate)
                scr = scr_pool.tile([128, CH], F32, tag="scr")
                nc.vector.scalar_tensor_tensor(
                    out=scr[0:m, :],
                    in0=e[0:m, :],
                    scalar=1.0,
                    in1=pds[c][0:m, :],
                    op0=ALU.mult,
                    op1=ALU.mult,
                    accum_out=nums[0:m, c : c + 1],
                )

            lp_cur = lp_next

            # merge chunks: M* = min_c mins ; s_c = exp(M* - mins_c)
            mstar = sm_pool.tile([128, 1], F32, tag="mstar")
            nc.vector.tensor_reduce(
                mstar[0:m], mins[0:m], axis=mybir.AxisListType.X, op=ALU.min
            )
            s = sm_pool.tile([128, NCH], F32, tag="s")
            nc.scalar.activation(
                s[0:m], mins[0:m], AF.Exp, bias=mstar[0:m], scale=-1.0
            )

            mscr = sm_pool.tile([128, NCH], F32, tag="mscr")
            dent = sm_pool.tile([128, 1], F32, tag="dent")
            nc.vector.scalar_tensor_tensor(
                out=mscr[0:m], in0=dens[0:m], scalar=1.0, in1=s[0:m],
                op0=ALU.mult, op1=ALU.mult, accum_out=dent[0:m],
            )
            mscr2 = sm_pool.tile([128, NCH], F32, tag="mscr2")
            numt = sm_pool.tile([128, 1], F32, tag="numt")
            nc.vector.scalar_tensor_tensor(
                out=mscr2[0:m], in0=nums[0:m], scalar=1.0, in1=s[0:m],
                op0=ALU.mult, op1=ALU.mult, accum_out=numt[0:m],
            )

            rec = sm_pool.tile([128, 1], F32, tag="rec")
            nc.vector.reciprocal(rec[0:m], dent[0:m])
            pcol = sm_pool.tile([128, 1], F32, tag="pcol")
            nc.vector.scalar_tensor_tensor(
                out=pcol[0:m, :],
                in0=numt[0:m, :],
                scalar=rec[0:m, :],
                in1=numt[0:m, :],
                op0=ALU.mult,
                op1=ALU.bypass,
            )
            nc.sync.dma_start(
                out[b : b + 1, off : off + m].rearrange("a b -> b a"), pcol[0:m, :]
            )


_NC = None


def _get_nc():
    global _NC
    if _NC is None:
        _NC = build_kernel_program()
    return _NC


def make_lmat():
    j = np.arange(128)[:, None]
    n = np.arange(128)[None, :]
    L = np.where(j < n, 0.5, 0.0).astype(np.float32)
    L[127, :] = 1.0
    return L


def make_in_maps(data, targets, W):
    data = np.ascontiguousarray(data, np.float32)
    targets = np.ascontiguousarray(targets, np.float32)
    W = np.ascontiguousarray(W, np.float32)
    wa = np.concatenate([W, W], axis=0)  # rows 64..127 duplicate W
    L = make_lmat()
    in_maps = []
    for c in range(NCORES):
        db = data[c * BL : (c + 1) * BL]       # (BL, N, D)
        tb = targets[c * BL : (c + 1) * BL]    # (BL, N)
        dt_ = db.transpose(0, 2, 1)
        dta = np.concatenate([dt_, dt_], axis=1)  # (BL, 2D, N) duplicated
        in_maps.append(
            {
                "dta": np.ascontiguousarray(dta, np.float32),
                "wa": wa,
                "lmat": L,
                "tcol": np.ascontiguousarray(-tb, np.float32),
            }
        )
    return in_maps


def kernel(data, targets, W):
    nc = _get_nc()
    in_maps = make_in_maps(data, targets, W)
    res = run_bass_kernel_spmd(nc, in_maps, list(range(NCORES)))
    outs = [res.results[c]["out"] for c in range(NCORES)]
    return np.concatenate(outs, axis=0).astype(np.float32)

